# revision 10
# baseline (speedup 1.0000x reference)
"""BiMamba block Trainium2 kernel (8 NeuronCores).

Sharding: core c -> batch b = c//4, role g = c%4.
 - Pre-scan phase: core handles time-quarter g of batch b (both directions).
 - AllGather (group of 4 = one batch) reshards delta/u/B/C/sz/sxd to full-T.
 - Scan phase: core handles state channels n in {4g..4g+3} over the FULL
   sequence (selective scan has no cross-core dependency this way).
 - Output partials (linear in the n-sum) combine via ReduceScatter.
Host folds LayerNorm gamma/beta into inproj and fuse_w@outw into one W2.
"""
import numpy as np

DIM = 512
DST = 16          # d_state
DI = 1024         # d_inner
DTR = 32          # dt_rank
BSZ, L = 2, 2048
T = 512           # quarter length
NCH = 8           # d-chunks of 128
NCORES = 8

_CACHE = {}


def _bf16(x):
    import ml_dtypes
    return np.asarray(x, dtype=np.float32).astype(ml_dtypes.bfloat16)


def _f16(x):
    return np.asarray(x, dtype=np.float16)


def _build_program():
    import concourse.bacc as bacc
    import concourse.mybir as mybir
    from concourse.tile import TileContext

    dt = mybir.dt
    AF = mybir.ActivationFunctionType
    OP = mybir.AluOpType

    nc = bacc.Bacc(num_devices=NCORES)

    # ---------------- I/O declarations ----------------
    ins = {}

    def inp(name, shape, dty):
        ins[name] = nc.dram_tensor(name, list(shape), dty, kind="ExternalInput")
        return ins[name]

    xf_in = inp("xf", (640, DIM), dt.float32)
    xb_in = inp("xb", (640, DIM), dt.float32)
    xres_in = inp("xres", (T, DIM), dt.float32)
    ident_in = inp("ident", (128, 128), dt.bfloat16)
    repsel_in = inp("repsel", (16, 4 * 128), dt.bfloat16)
    for d in ("f", "b"):
        inp(f"{d}_inT", (DIM, 2 * DI), dt.bfloat16)
        inp(f"{d}_inb", (128, 16), dt.float32)
        inp(f"{d}_convw", (128, NCH * 4), dt.float32)
        inp(f"{d}_convb", (128, NCH), dt.float32)
        inp(f"{d}_xprojT", (DI, DTR + 2 * DST), dt.bfloat16)
        inp(f"{d}_dtwT", (DTR, DI), dt.bfloat16)
        inp(f"{d}_dtb", (128, NCH), dt.float32)
        inp(f"{d}_ascale", (128, NCH * 4), dt.float32)
        inp(f"{d}_D", (128, NCH), dt.float32)
        inp(f"{d}_W2T", (DI, DIM), dt.bfloat16)
    out_t = nc.dram_tensor("out", [T, DIM], dt.float32, kind="ExternalOutput")

    # collective DRAM tensors
    ag_d_in = nc.dram_tensor("ag_d_in", [256, NCH * T], dt.float32, kind="Internal")
    ag_d_out = nc.dram_tensor("ag_d_out", [1024, NCH * T], dt.float32, kind="Internal")
    ag_u_in = nc.dram_tensor("ag_u_in", [256, NCH * T], dt.bfloat16, kind="Internal")
    ag_u_out = nc.dram_tensor("ag_u_out", [1024, NCH * T], dt.bfloat16, kind="Internal")
    ag_bc_in = nc.dram_tensor("ag_bc_in", [64, T], dt.bfloat16, kind="Internal")
    ag_bc_out = nc.dram_tensor("ag_bc_out", [256, T], dt.bfloat16, kind="Internal")
    ag_g_in = nc.dram_tensor("ag_g_in", [512, NCH * T], dt.bfloat16, kind="Internal")
    ag_g_out = nc.dram_tensor("ag_g_out", [2048, NCH * T], dt.bfloat16, kind="Internal")
    rs_in = nc.dram_tensor("rs_in", [L, DIM], dt.float32, kind="Internal")
    rs_out = nc.dram_tensor("rs_out", [T, DIM], dt.float32, kind="Internal")
    groups = [[0, 1, 2, 3], [4, 5, 6, 7]]

    with TileContext(nc) as tc:
        with tc.tile_pool(name="persist", bufs=1) as pp:
            ident = pp.tile([128, 128], dt.bfloat16)
            nc.sync.dma_start(ident[:, :], ident_in[:, :])
            repsel = pp.tile([16, 4 * 128], dt.bfloat16)
            nc.sync.dma_start(repsel[:, :], repsel_in[:, :])
            # yg big tiles persist until the output matmul
            yg = {}
            for d in ("f", "b"):
                for ch in range(NCH):
                    yg[(d, ch)] = pp.tile([128, L], dt.bfloat16, tag=f"yg{d}{ch}", name=f"yg{d}{ch}")

            # ================= pre-scan phase (per direction) ==========
            for d in ("f", "b"):
                x_in = xf_in if d == "f" else xb_in
                with tc.tile_pool(name=f"pre{d}", bufs=1) as ppre, \
                     tc.tile_pool(name=f"pre{d}w", bufs=2) as ppw, \
                     tc.tile_pool(name=f"ps{d}a", bufs=2, space="PSUM") as psA, \
                     tc.tile_pool(name=f"ps{d}b", bufs=1, space="PSUM") as psB:
                    # --- load x tiles, LayerNorm in [t,k], transpose ---
                    xnT = [ppre.tile([128, 640], dt.bfloat16, tag=f"xnT{j}", name=f"xnT{d}{j}")
                           for j in range(4)]
                    for i in range(5):
                        xt = ppw.tile([128, DIM], dt.float32, tag="xt")
                        nc.sync.dma_start(xt[:, :], x_in[i * 128:(i + 1) * 128, :])
                        sq = ppw.tile([128, DIM], dt.float32, tag="sq")
                        nc.scalar.activation(sq[:, :], xt[:, :], AF.Square)
                        st = ppw.tile([128, 8], dt.float32, tag="st")
                        nc.vector.tensor_reduce(st[:, 0:1], xt[:, :],
                                                mybir.AxisListType.X, OP.add)
                        nc.vector.tensor_reduce(st[:, 1:2], sq[:, :],
                                                mybir.AxisListType.X, OP.add)
                        # nmu = -sum/512 ; m2 = sumsq/512
                        nc.vector.tensor_scalar(st[:, 2:3], st[:, 0:1],
                                                -1.0 / DIM, None, OP.mult)
                        nc.vector.tensor_scalar(st[:, 3:4], st[:, 1:2],
                                                1.0 / DIM, None, OP.mult)
                        # var = m2 - mu^2 + eps
                        nc.vector.tensor_tensor(st[:, 4:5], st[:, 2:3], st[:, 2:3],
                                                OP.mult)
                        nc.vector.tensor_tensor(st[:, 5:6], st[:, 3:4], st[:, 4:5],
                                                OP.subtract)
                        nc.vector.tensor_scalar(st[:, 5:6], st[:, 5:6], 1e-5, None,
                                                OP.add)
                        nc.vector.reciprocal(st[:, 6:7], st[:, 5:6])
                        nc.scalar.activation(st[:, 7:8], st[:, 6:7], AF.Sqrt)
                        xn = ppw.tile([128, DIM], dt.bfloat16, tag="xn")
                        nc.vector.tensor_scalar(xn[:, :], xt[:, :], st[:, 2:3],
                                                st[:, 7:8], OP.add, op1=OP.mult)
                        # transpose 128x128 blocks into xnT big tiles
                        for j in range(4):
                            tp = psA.tile([128, 128], dt.bfloat16, tag="tp")
                            nc.tensor.transpose(tp[:, :], xn[:, j * 128:(j + 1) * 128],
                                                ident[:, :])
                            nc.scalar.activation(
                                xnT[j][:, i * 128:(i + 1) * 128], tp[:, :], AF.Copy)

                    # --- inproj matmul -> xc chunks + z big tile ---
                    z_big = ppre.tile([128, NCH * T], dt.bfloat16, tag="zbig")
                    xc = [ppre.tile([128, 515], dt.bfloat16, tag=f"xc{m}", name=f"xc{d}{m}")
                          for m in range(NCH)]
                    inb = ppw.tile([128, 16], dt.float32, tag="inb")
                    nc.sync.dma_start(inb[:, :], ins[f"{d}_inb"][:, :])
                    wk_tiles = []
                    for k in range(4):
                        wk = ppw.tile([128, 2 * DI], dt.bfloat16, tag=f"winT{k}")
                        nc.sync.dma_start(
                            wk[:, :], ins[f"{d}_inT"][k * 128:(k + 1) * 128, :])
                        wk_tiles.append(wk)
                    for m in range(16):
                        ps = psA.tile([128, 1024], dt.float32, tag="xzps")
                        for k in range(4):
                            lhsT = wk_tiles[k][:, m * 128:(m + 1) * 128]
                            if m < NCH:
                                nc.tensor.matmul(ps[:, 0:512], lhsT,
                                                 xnT[k][:, 0:512],
                                                 start=(k == 0), stop=(k == 3))
                                nc.tensor.matmul(ps[:, 512:515], lhsT,
                                                 xnT[k][:, 512:515],
                                                 start=(k == 0), stop=(k == 3))
                            else:
                                nc.tensor.matmul(ps[:, 0:512], lhsT,
                                                 xnT[k][:, 3:515],
                                                 start=(k == 0), stop=(k == 3))
                        if m < NCH:
                            nc.scalar.activation(xc[m][:, 0:512], ps[:, 0:512],
                                                 AF.Identity,
                                                 bias=inb[:, m:m + 1])
                            nc.scalar.activation(xc[m][:, 512:515], ps[:, 512:515],
                                                 AF.Identity,
                                                 bias=inb[:, m:m + 1])
                        else:
                            mm = m - NCH
                            nc.scalar.activation(
                                z_big[:, mm * T:(mm + 1) * T], ps[:, 0:512],
                                AF.Identity, bias=inb[:, m:m + 1])

                    # --- causal conv (4 taps) + silu ---
                    convw = ppw.tile([128, NCH * 4], dt.float32, tag="convw")
                    convb = ppw.tile([128, NCH], dt.float32, tag="convb")
                    nc.sync.dma_start(convw[:, :], ins[f"{d}_convw"][:, :])
                    nc.sync.dma_start(convb[:, :], ins[f"{d}_convb"][:, :])
                    cv_big = ppre.tile([128, NCH * T], dt.bfloat16, tag="cvbig")
                    for ch in range(NCH):
                        sl = cv_big[:, ch * T:(ch + 1) * T]
                        nc.vector.tensor_scalar(
                            sl, xc[ch][:, 0:512], convw[:, ch * 4:ch * 4 + 1],
                            convb[:, ch:ch + 1], OP.mult, op1=OP.add)
                        for k in range(1, 4):
                            nc.vector.scalar_tensor_tensor(
                                sl, xc[ch][:, k:k + 512],
                                convw[:, ch * 4 + k:ch * 4 + k + 1], sl,
                                OP.mult, OP.add)
                    sx_big = ppre.tile([128, NCH * T], dt.bfloat16, tag="sxbig")
                    nc.scalar.activation(sx_big[:, :], cv_big[:, :], AF.Silu)
                    sz_big = ppre.tile([128, NCH * T], dt.bfloat16, tag="szbig")
                    nc.scalar.activation(sz_big[:, :], z_big[:, :], AF.Silu)

                    # --- xproj -> dbc (dt | B | C) ---
                    xprojT = ppw.tile([128, NCH * (DTR + 2 * DST)], dt.bfloat16,
                                      tag="xprojT")
                    # stored as 8 chunks of [128, 64]
                    for ch in range(NCH):
                        nc.sync.dma_start(
                            xprojT[:, ch * 64:(ch + 1) * 64],
                            ins[f"{d}_xprojT"][ch * 128:(ch + 1) * 128, :])
                    dbc_ps = psB.tile([64, 512], dt.float32, tag="dbcps")
                    for ch in range(NCH):
                        nc.tensor.matmul(
                            dbc_ps[:, :], xprojT[:, ch * 64:(ch + 1) * 64],
                            sx_big[:, ch * T:(ch + 1) * T],
                            start=(ch == 0), stop=(ch == NCH - 1))
                    dt_sb = ppw.tile([DTR, 512], dt.bfloat16, tag="dtsb")
                    nc.scalar.activation(dt_sb[:, :], dbc_ps[0:DTR, :], AF.Copy)
                    bc_sb = ppw.tile([32, 512], dt.bfloat16, tag="bcsb")
                    nc.scalar.activation(bc_sb[:, :], dbc_ps[DTR:64, :], AF.Copy)
                    # stage B/C rows to ag_bc_in ([64, T]: dir*32 + {B16|C16})
                    off = 0 if d == "f" else 32
                    nc.sync.dma_start(ag_bc_in[off:off + 32, :], bc_sb[:, :])

                    # --- dt proj + softplus -> delta (fp16) ---
                    dtwT = ppw.tile([DTR, DI], dt.bfloat16, tag="dtwT")
                    nc.sync.dma_start(dtwT[:, :], ins[f"{d}_dtwT"][:, :])
                    dtb = ppw.tile([128, NCH], dt.float32, tag="dtb")
                    nc.sync.dma_start(dtb[:, :], ins[f"{d}_dtb"][:, :])
                    del_big = ppre.tile([128, NCH * T], dt.float32, tag="delbig")
                    for ch in range(NCH):
                        dp = psB.tile([128, 512], dt.float32, tag="dpps")
                        nc.tensor.matmul(dp[:, :],
                                         dtwT[:, ch * 128:(ch + 1) * 128],
                                         dt_sb[:, :], start=True, stop=True)
                        et = ppw.tile([128, 512], dt.float32, tag="et")
                        nc.scalar.activation(et[:, :], dp[:, :], AF.Exp,
                                             bias=dtb[:, ch:ch + 1])
                        nc.scalar.activation(del_big[:, ch * T:(ch + 1) * T],
                                             et[:, :], AF.Ln, bias=1.0)
                    # u = delta * silu(xconv)   (bf16)
                    u_big = ppre.tile([128, NCH * T], dt.bfloat16, tag="ubig")
                    nc.vector.tensor_tensor(u_big[:, :], del_big[:, :],
                                            sx_big[:, :], OP.mult)
                    # sxd = sx * D * 0.25  (bf16)
                    Dt = ppw.tile([128, NCH], dt.float32, tag="Dt")
                    nc.sync.dma_start(Dt[:, :], ins[f"{d}_D"][:, :])
                    sxd_big = ppre.tile([128, NCH * T], dt.bfloat16, tag="sxdbig")
                    for ch in range(NCH):
                        nc.vector.tensor_scalar(
                            sxd_big[:, ch * T:(ch + 1) * T],
                            sx_big[:, ch * T:(ch + 1) * T],
                            Dt[:, ch:ch + 1], 0.25, OP.mult, op1=OP.mult)

                    # --- stage AG payloads ---
                    roff = 0 if d == "f" else 128
                    nc.sync.dma_start(ag_d_in[roff:roff + 128, :], del_big[:, :])
                    nc.sync.dma_start(ag_u_in[roff:roff + 128, :], u_big[:, :])
                    goff = 0 if d == "f" else 256
                    nc.sync.dma_start(ag_g_in[goff:goff + 128, :], sz_big[:, :])
                    nc.sync.dma_start(ag_g_in[goff + 128:goff + 256, :],
                                      sxd_big[:, :])

            # ================= collectives: reshard ====================
            OPb = mybir.AluOpType.bypass
            nc.gpsimd.collective_compute("AllGather", OPb, replica_groups=groups,
                                         ins=[ag_bc_in[:, :]],
                                         outs=[ag_bc_out[:, :]])
            nc.gpsimd.collective_compute("AllGather", OPb, replica_groups=groups,
                                         ins=[ag_d_in[:, :]],
                                         outs=[ag_d_out[:, :]])
            nc.gpsimd.collective_compute("AllGather", OPb, replica_groups=groups,
                                         ins=[ag_u_in[:, :]],
                                         outs=[ag_u_out[:, :]])
            nc.gpsimd.collective_compute("AllGather", OPb, replica_groups=groups,
                                         ins=[ag_g_in[:, :]],
                                         outs=[ag_g_out[:, :]])

            # ================= scan phase (my 4 n's, full T) ===========
            for d in ("f", "b"):
                roff = 0 if d == "f" else 128
                boff = 0 if d == "f" else 32
                with tc.tile_pool(name=f"scan{d}", bufs=1) as psc, \
                     tc.tile_pool(name=f"scw{d}", bufs=2) as pscw:
                    # full-T B^T / C^T ([16, L]) assembled from quarters
                    BT = psc.tile([16, L], dt.bfloat16, tag="BT")
                    CT = psc.tile([16, L], dt.bfloat16, tag="CT")
                    for qq in range(4):
                        nc.sync.dma_start(
                            BT[:, qq * T:(qq + 1) * T],
                            ag_bc_out[qq * 64 + boff:qq * 64 + boff + 16, :])
                        nc.sync.dma_start(
                            CT[:, qq * T:(qq + 1) * T],
                            ag_bc_out[qq * 64 + boff + 16:qq * 64 + boff + 32, :])
                    # replicate my 4 n-rows across partitions (PE)
                    Brep, Crep = [], []
                    with tc.tile_pool(name=f"rp{d}", bufs=2,
                                      space="PSUM") as prep:
                        for ni in range(4):
                            for (srct, dst) in ((BT, Brep), (CT, Crep)):
                                rp = psc.tile(
                                    [128, L], dt.bfloat16,
                                    tag=f"rep{'B' if dst is Brep else 'C'}{ni}",
                                    name=f"rep{d}{ni}{len(dst)}")
                                for fc in range(4):
                                    pr = prep.tile([128, 512], dt.float32,
                                                   tag="repps")
                                    nc.tensor.matmul(
                                        pr[:, :],
                                        repsel[:, ni * 128:(ni + 1) * 128],
                                        srct[:, fc * 512:(fc + 1) * 512],
                                        start=True, stop=True)
                                    nc.vector.tensor_copy(
                                        rp[:, fc * 512:(fc + 1) * 512], pr[:, :])
                                dst.append(rp)
                    ascale = psc.tile([128, NCH * 4], dt.float32, tag="ascale")
                    nc.sync.dma_start(ascale[:, :], ins[f"{d}_ascale"][:, :])
                    pyps_cm = tc.tile_pool(name=f"yps{d}", bufs=2,
                                           space="PSUM")
                    pyps = pyps_cm.__enter__()
                    for ch in range(NCH):
                        # stream full-T delta / u / sz / sxd for this chunk
                        dl = pscw.tile([128, L], dt.float32, tag="dl")
                        uu = pscw.tile([128, L], dt.bfloat16, tag="uu")
                        sz = pscw.tile([128, L], dt.bfloat16, tag="sz")
                        sxd = pscw.tile([128, L], dt.bfloat16, tag="sxd")
                        for qq in range(4):
                            r0 = qq * 256 + roff
                            g0 = qq * 512 + (0 if d == "f" else 256)
                            cs = slice(ch * T, (ch + 1) * T)
                            ts_ = slice(qq * T, (qq + 1) * T)
                            nc.sync.dma_start(dl[:, ts_], ag_d_out[r0:r0 + 128, cs])
                            nc.sync.dma_start(uu[:, ts_], ag_u_out[r0:r0 + 128, cs])
                            nc.sync.dma_start(sz[:, ts_],
                                              ag_g_out[g0:g0 + 128, cs])
                            nc.sync.dma_start(sxd[:, ts_],
                                              ag_g_out[g0 + 128:g0 + 256, cs])
                        y_ps = pyps.tile([128, L], dt.float32, tag="yps")
                        for ni in range(4):
                            dA = pscw.tile([128, L], dt.float32, tag="dA")
                            nc.scalar.activation(dA[:, :], dl[:, :], AF.Exp,
                                                 scale=ascale[:, ch * 4 + ni:
                                                              ch * 4 + ni + 1])
                            dBx = pscw.tile([128, L], dt.bfloat16, tag="dBx")
                            nc.vector.tensor_tensor(dBx[:, :], uu[:, :],
                                                    Brep[ni][:, :], OP.mult)
                            hh = pscw.tile([128, L], dt.bfloat16, tag="hh")
                            nc.vector.tensor_tensor_scan(
                                hh[:, :], dA[:, :], dBx[:, :], 0.0,
                                OP.mult, OP.add)
                            hC = pscw.tile([128, L], dt.bfloat16, tag="hC")
                            nc.vector.tensor_tensor(hC[:, :], hh[:, :],
                                                    Crep[ni][:, :], OP.mult)
                            for fc in range(4):
                                nc.tensor.matmul(
                                    y_ps[:, fc * 512:(fc + 1) * 512],
                                    ident[:, :], hC[:, fc * 512:(fc + 1) * 512],
                                    start=(ni == 0), stop=False)
                        # + sxd/4 via identity matmul, then gate with silu(z)
                        for fc in range(4):
                            nc.tensor.matmul(
                                y_ps[:, fc * 512:(fc + 1) * 512], ident[:, :],
                                sxd[:, fc * 512:(fc + 1) * 512],
                                start=False, stop=True)
                        dst = yg[(d, ch)][:, :]
                        if d == "b":
                            dst = dst[:, ::-1]
                        nc.vector.tensor_tensor(dst, y_ps[:, :], sz[:, :], OP.mult)
                    pyps_cm.__exit__(None, None, None)

            # ================= output matmul + ReduceScatter ===========
            with tc.tile_pool(name="outp", bufs=2) as pout, \
                 tc.tile_pool(name="outps", bufs=2, space="PSUM") as pops:
                W2 = {}
                for d in ("f", "b"):
                    W2[d] = pout.tile([128, NCH * DIM], dt.bfloat16, tag=f"W2{d}", name=f"W2{d}")
                    for ch in range(NCH):
                        nc.sync.dma_start(
                            W2[d][:, ch * DIM:(ch + 1) * DIM],
                            ins[f"{d}_W2T"][ch * 128:(ch + 1) * 128, :])
                for m in range(16):
                    ps = pops.tile([128, DIM], dt.float32, tag="ops")
                    first = True
                    for d in ("f", "b"):
                        for ch in range(NCH):
                            nc.tensor.matmul(
                                ps[:, :],
                                yg[(d, ch)][:, m * 128:(m + 1) * 128],
                                W2[d][:, ch * DIM:(ch + 1) * DIM],
                                start=first, stop=(d == "b" and ch == NCH - 1))
                            first = False
                    ob = pout.tile([128, DIM], dt.float32, tag="ob")
                    nc.vector.tensor_copy(ob[:, :], ps[:, :])
                    nc.sync.dma_start(rs_in[m * 128:(m + 1) * 128, :], ob[:, :])
                nc.gpsimd.collective_compute(
                    "ReduceScatter", mybir.AluOpType.add, replica_groups=groups,
                    ins=[rs_in[:, :]], outs=[rs_out[:, :]])
                for i in range(4):
                    rt = pout.tile([128, DIM], dt.float32, tag="rt")
                    nc.sync.dma_start(rt[:, :], rs_out[i * 128:(i + 1) * 128, :])
                    xr = pout.tile([128, DIM], dt.float32, tag="xr")
                    nc.sync.dma_start(xr[:, :], xres_in[i * 128:(i + 1) * 128, :])
                    oo = pout.tile([128, DIM], dt.float32, tag="oo")
                    nc.vector.tensor_tensor(oo[:, :], rt[:, :], xr[:, :], OP.add)
                    nc.sync.dma_start(out_t[i * 128:(i + 1) * 128, :], oo[:, :])

    nc.compile()
    return nc


def _prep_host(inputs):
    """Fold weights, build the 8 per-core input maps."""
    f32 = np.float32
    x = np.asarray(inputs["x"], f32)
    ln_g = np.asarray(inputs["ln_g"], f32)
    ln_b = np.asarray(inputs["ln_b"], f32)
    fuse_w = np.asarray(inputs["fuse_w"], f32)
    fuse_b = np.asarray(inputs["fuse_b"], f32)

    shared = {}
    shared["ident"] = _bf16(np.eye(128))
    for d in ("f", "b"):
        P = {k[2:]: np.asarray(v, f32) for k, v in inputs.items()
             if k.startswith(d + "_")}
        inW = P["inproj"] * ln_g[None, :]
        shared[f"{d}_inT"] = _bf16(inW.T)                       # [512, 2048]
        shared[f"{d}_inb"] = (P["inproj"] @ ln_b).reshape(16, 128).T.copy()
        shared[f"{d}_convw"] = P["convw"].reshape(NCH, 128, 4).transpose(
            1, 0, 2).reshape(128, NCH * 4).copy()
        shared[f"{d}_convb"] = P["convb"].reshape(NCH, 128).T.copy()
        shared[f"{d}_xprojT"] = _bf16(P["xproj"].T)             # [1024, 64]
        shared[f"{d}_dtwT"] = _bf16(P["dtw"].T)                 # [32, 1024]
        shared[f"{d}_dtb"] = P["dtb"].reshape(NCH, 128).T.copy()
        shared[f"{d}_D"] = P["D"].reshape(NCH, 128).T.copy()
        shared[f"{d}_W2T"] = _bf16((fuse_w @ P["outw"]).T)      # [1024, 512]
        shared[f"{d}_A"] = -np.exp(P["Alog"])                   # [1024, 16]

    xr_full = x[:, ::-1, :]
    maps = []
    for c in range(NCORES):
        b, g = c // 4, c % 4
        m = {k: v for k, v in shared.items() if not k.endswith("_A")}

        def pad_slice(src):
            lo = g * T - 3
            sl = np.zeros((640, DIM), f32)
            a = max(lo, 0)
            sl[a - lo:(g + 1) * T - lo, :] = src[a:(g + 1) * T, :]
            return sl

        m["xf"] = pad_slice(x[b])
        m["xb"] = pad_slice(xr_full[b])
        m["xres"] = (x[b, g * T:(g + 1) * T, :] + fuse_b[None, :]).astype(f32)
        rs = np.zeros((16, 4 * 128), f32)
        for ni in range(4):
            rs[4 * g + ni, ni * 128:(ni + 1) * 128] = 1.0
        m["repsel"] = _bf16(rs)
        for d in ("f", "b"):
            A = shared[f"{d}_A"]  # [1024, 16]
            asc = np.zeros((128, NCH * 4), f32)
            for ch in range(NCH):
                for ni in range(4):
                    asc[:, ch * 4 + ni] = A[ch * 128:(ch + 1) * 128, 4 * g + ni]
            m[f"{d}_ascale"] = asc
        m["xf"] = m["xf"].astype(f32)
        maps.append(m)
    return maps


def kernel(**inputs):
    from concourse.bass_utils import run_bass_kernel_spmd

    if "nc" not in _CACHE:
        _CACHE["nc"] = _build_program()
    nc = _CACHE["nc"]

    maps = _prep_host(inputs)
    res = run_bass_kernel_spmd(nc, maps, core_ids=list(range(NCORES)))
    out = np.zeros((BSZ, L, DIM), np.float32)
    for c in range(NCORES):
        b, g = c // 4, c % 4
        out[b, g * T:(g + 1) * T, :] = res.results[c]["out"]
    return out


# revision 18
# speedup vs baseline: 1.1809x; 1.1809x over previous
"""BiMamba block Trainium2 kernel (8 NeuronCores).

Sharding: core c -> batch b = c//4, role g = c%4.
 - Pre-scan phase: core handles time-quarter g of batch b (both directions).
 - AllGather (group of 4 = one batch) reshards delta/u/B/C/sz/sxd to full-T.
 - Scan phase: core handles state channels n in {4g..4g+3} over the FULL
   sequence (selective scan has no cross-core dependency this way).
 - Output partials (linear in the n-sum) combine via ReduceScatter.
Host folds LayerNorm gamma/beta into inproj and fuse_w@outw into one W2.
"""
import numpy as np

DIM = 512
DST = 16          # d_state
DI = 1024         # d_inner
DTR = 32          # dt_rank
BSZ, L = 2, 2048
T = 512           # quarter length
NCH = 8           # d-chunks of 128
NCORES = 8

_CACHE = {}


def _bf16(x):
    import ml_dtypes
    return np.asarray(x, dtype=np.float32).astype(ml_dtypes.bfloat16)


def _build_program():
    import concourse.bacc as bacc
    import concourse.mybir as mybir
    from concourse.tile import TileContext

    dt = mybir.dt
    AF = mybir.ActivationFunctionType
    OP = mybir.AluOpType

    nc = bacc.Bacc(num_devices=NCORES)

    # ---------------- I/O declarations ----------------
    ins = {}

    def inp(name, shape, dty):
        ins[name] = nc.dram_tensor(name, list(shape), dty, kind="ExternalInput")
        return ins[name]

    xf_in = inp("xf", (640, DIM), dt.float32)
    xb_in = inp("xb", (640, DIM), dt.float32)
    xres_in = inp("xres", (T, DIM), dt.float32)
    ident_in = inp("ident", (128, 128), dt.bfloat16)
    repsel_in = inp("repsel", (16, 4 * 128), dt.bfloat16)
    for d in ("f", "b"):
        inp(f"{d}_inT", (DIM, 2 * DI), dt.bfloat16)
        inp(f"{d}_inb", (128, 16), dt.float32)
        inp(f"{d}_convw", (128, NCH * 4), dt.float32)
        inp(f"{d}_convb", (128, NCH), dt.float32)
        inp(f"{d}_xprojT", (DI, DTR + 2 * DST), dt.bfloat16)
        inp(f"{d}_dtwT", (DTR, DI), dt.bfloat16)
        inp(f"{d}_dtb", (128, NCH), dt.float32)
        inp(f"{d}_ascale", (128, NCH * 4), dt.float32)
        inp(f"{d}_D", (128, NCH), dt.float32)
        inp(f"{d}_W2T", (DI, DIM), dt.bfloat16)
    out_t = nc.dram_tensor("out", [T, DIM], dt.float32, kind="ExternalOutput")

    # collective DRAM tensors
    agt = {}
    for d in ("f", "b"):
        agt[f"d_in_{d}"] = nc.dram_tensor(f"ag_d_in_{d}", [128, NCH * T],
                                          dt.float16, kind="Internal")
        agt[f"d_out_{d}"] = nc.dram_tensor(f"ag_d_out_{d}", [512, NCH * T],
                                           dt.float16, kind="Internal")
        agt[f"u_in_{d}"] = nc.dram_tensor(f"ag_u_in_{d}", [128, NCH * T],
                                          dt.bfloat16, kind="Internal")
        agt[f"u_out_{d}"] = nc.dram_tensor(f"ag_u_out_{d}", [512, NCH * T],
                                           dt.bfloat16, kind="Internal")
        agt[f"bc_in_{d}"] = nc.dram_tensor(f"ag_bc_in_{d}", [32, T],
                                           dt.bfloat16, kind="Internal")
        agt[f"bc_out_{d}"] = nc.dram_tensor(f"ag_bc_out_{d}", [128, T],
                                            dt.bfloat16, kind="Internal")
        agt[f"g_in_{d}"] = nc.dram_tensor(f"ag_g_in_{d}", [256, NCH * T],
                                          dt.bfloat16, kind="Internal")
        agt[f"g_out_{d}"] = nc.dram_tensor(f"ag_g_out_{d}", [1024, NCH * T],
                                           dt.bfloat16, kind="Internal")
    rs_in = nc.dram_tensor("rs_in", [L, DIM], dt.float32, kind="Internal")
    rs_out = nc.dram_tensor("rs_out", [T, DIM], dt.float32, kind="Internal")
    groups = [[0, 1, 2, 3], [4, 5, 6, 7]]

    with TileContext(nc) as tc:
        with tc.tile_pool(name="persist", bufs=1) as pp:
            ident = pp.tile([128, 128], dt.bfloat16)
            nc.sync.dma_start(ident[:, :], ident_in[:, :])
            repsel = pp.tile([16, 4 * 128], dt.bfloat16)
            nc.sync.dma_start(repsel[:, :], repsel_in[:, :])
            # ================= pre-scan phase (per direction) ==========
            with tc.tile_pool(name="pre", bufs=1) as ppre, \
                 tc.tile_pool(name="prew", bufs=2) as ppw, \
                 tc.tile_pool(name="psa", bufs=2, space="PSUM") as psA, \
                 tc.tile_pool(name="psb", bufs=1, space="PSUM") as psB:
              for d in ("f", "b"):
                x_in = xf_in if d == "f" else xb_in
                if True:
                    # --- load x tiles, LayerNorm in [t,k], transpose ---
                    xnT = [ppre.tile([128, 640], dt.bfloat16, tag=f"xnT{d}{j}", name=f"xnT{d}{j}")
                           for j in range(4)]
                    for i in range(5):
                        xt = ppw.tile([128, DIM], dt.float32, tag="xt")
                        nc.sync.dma_start(xt[:, :], x_in[i * 128:(i + 1) * 128, :])
                        sq = ppw.tile([128, DIM], dt.float32, tag="sq")
                        nc.scalar.activation(sq[:, :], xt[:, :], AF.Square)
                        st = ppw.tile([128, 8], dt.float32, tag="st")
                        nc.vector.tensor_reduce(st[:, 0:1], xt[:, :],
                                                mybir.AxisListType.X, OP.add)
                        nc.vector.tensor_reduce(st[:, 1:2], sq[:, :],
                                                mybir.AxisListType.X, OP.add)
                        # nmu = -sum/512 ; m2 = sumsq/512
                        nc.vector.tensor_scalar(st[:, 2:3], st[:, 0:1],
                                                -1.0 / DIM, None, OP.mult)
                        nc.vector.tensor_scalar(st[:, 3:4], st[:, 1:2],
                                                1.0 / DIM, None, OP.mult)
                        # var = m2 - mu^2 + eps
                        nc.vector.tensor_tensor(st[:, 4:5], st[:, 2:3], st[:, 2:3],
                                                OP.mult)
                        nc.vector.tensor_tensor(st[:, 5:6], st[:, 3:4], st[:, 4:5],
                                                OP.subtract)
                        nc.vector.tensor_scalar(st[:, 5:6], st[:, 5:6], 1e-5, None,
                                                OP.add)
                        nc.vector.reciprocal(st[:, 6:7], st[:, 5:6])
                        nc.scalar.activation(st[:, 7:8], st[:, 6:7], AF.Sqrt)
                        xn = ppw.tile([128, DIM], dt.bfloat16, tag="xn")
                        nc.vector.tensor_scalar(xn[:, :], xt[:, :], st[:, 2:3],
                                                st[:, 7:8], OP.add, op1=OP.mult)
                        # transpose 128x128 blocks into xnT big tiles
                        for j in range(4):
                            tp = psA.tile([128, 128], dt.bfloat16, tag="tp")
                            nc.tensor.transpose(tp[:, :], xn[:, j * 128:(j + 1) * 128],
                                                ident[:, :])
                            nc.scalar.activation(
                                xnT[j][:, i * 128:(i + 1) * 128], tp[:, :], AF.Copy)

                    # --- inproj matmul -> xc chunks + z big tile ---
                    z_big = ppre.tile([128, NCH * T], dt.bfloat16, tag="zbig")
                    xc = [ppre.tile([128, 515], dt.bfloat16, tag=f"xc{m}", name=f"xc{d}{m}")
                          for m in range(NCH)]
                    inb = ppw.tile([128, 16], dt.float32, tag="inb")
                    nc.sync.dma_start(inb[:, :], ins[f"{d}_inb"][:, :])
                    wk_tiles = []
                    for k in range(4):
                        wk = ppre.tile([128, 2 * DI], dt.bfloat16, tag=f"winT{k}", name=f"winT{d}{k}")
                        nc.sync.dma_start(
                            wk[:, :], ins[f"{d}_inT"][k * 128:(k + 1) * 128, :])
                        wk_tiles.append(wk)
                    for m in range(16):
                        ps = psA.tile([128, 1024], dt.float32, tag="xzps")
                        for k in range(4):
                            lhsT = wk_tiles[k][:, m * 128:(m + 1) * 128]
                            if m < NCH:
                                nc.tensor.matmul(ps[:, 0:512], lhsT,
                                                 xnT[k][:, 0:512],
                                                 start=(k == 0), stop=(k == 3))
                                nc.tensor.matmul(ps[:, 512:515], lhsT,
                                                 xnT[k][:, 512:515],
                                                 start=(k == 0), stop=(k == 3))
                            else:
                                nc.tensor.matmul(ps[:, 0:512], lhsT,
                                                 xnT[k][:, 3:515],
                                                 start=(k == 0), stop=(k == 3))
                        if m < NCH:
                            nc.vector.tensor_scalar(xc[m][:, 0:512], ps[:, 0:512],
                                                    inb[:, m:m + 1], None, OP.add)
                            nc.vector.tensor_scalar(xc[m][:, 512:515],
                                                    ps[:, 512:515],
                                                    inb[:, m:m + 1], None, OP.add)
                        else:
                            mm = m - NCH
                            nc.scalar.activation(
                                z_big[:, mm * T:(mm + 1) * T], ps[:, 0:512],
                                AF.Identity, bias=inb[:, m:m + 1])

                    # --- causal conv (4 taps) + silu ---
                    convw = ppw.tile([128, NCH * 4], dt.float32, tag="convw")
                    convb = ppw.tile([128, NCH], dt.float32, tag="convb")
                    nc.sync.dma_start(convw[:, :], ins[f"{d}_convw"][:, :])
                    nc.sync.dma_start(convb[:, :], ins[f"{d}_convb"][:, :])
                    cv_big = ppre.tile([128, NCH * T], dt.bfloat16, tag="cvbig")
                    for ch in range(NCH):
                        sl = cv_big[:, ch * T:(ch + 1) * T]
                        nc.vector.tensor_scalar(
                            sl, xc[ch][:, 0:512], convw[:, ch * 4:ch * 4 + 1],
                            convb[:, ch:ch + 1], OP.mult, op1=OP.add)
                        for k in range(1, 4):
                            nc.vector.scalar_tensor_tensor(
                                sl, xc[ch][:, k:k + 512],
                                convw[:, ch * 4 + k:ch * 4 + k + 1], sl,
                                OP.mult, OP.add)
                    sx_big = ppre.tile([128, NCH * T], dt.bfloat16, tag="sxbig" + d)
                    nc.scalar.activation(sx_big[:, :], cv_big[:, :], AF.Silu)
                    sz_big = ppre.tile([128, NCH * T], dt.bfloat16, tag="szbig" + d)
                    nc.scalar.activation(sz_big[:, :], z_big[:, :], AF.Silu)

                    # --- xproj -> dbc (dt | B | C) ---
                    xprojT = ppw.tile([128, NCH * (DTR + 2 * DST)], dt.bfloat16,
                                      tag="xprojT")
                    # stored as 8 chunks of [128, 64]
                    for ch in range(NCH):
                        nc.sync.dma_start(
                            xprojT[:, ch * 64:(ch + 1) * 64],
                            ins[f"{d}_xprojT"][ch * 128:(ch + 1) * 128, :])
                    dbc_ps = psB.tile([64, 512], dt.float32, tag="dbcps")
                    for ch in range(NCH):
                        nc.tensor.matmul(
                            dbc_ps[:, :], xprojT[:, ch * 64:(ch + 1) * 64],
                            sx_big[:, ch * T:(ch + 1) * T],
                            start=(ch == 0), stop=(ch == NCH - 1))
                    dt_sb = ppw.tile([DTR, 512], dt.bfloat16, tag="dtsb")
                    nc.scalar.activation(dt_sb[:, :], dbc_ps[0:DTR, :], AF.Copy)
                    bc_sb = ppw.tile([32, 512], dt.bfloat16, tag="bcsb")
                    nc.scalar.activation(bc_sb[:, :], dbc_ps[DTR:64, :], AF.Copy)
                    # stage B/C rows to ag_bc_in ([64, T]: dir*32 + {B16|C16})
                    nc.sync.dma_start(agt[f"bc_in_{d}"][:, :], bc_sb[:, :])

                    # --- dt proj + softplus -> delta (fp16) ---
                    dtwT = ppw.tile([DTR, DI], dt.bfloat16, tag="dtwT")
                    nc.sync.dma_start(dtwT[:, :], ins[f"{d}_dtwT"][:, :])
                    dtb = ppw.tile([128, NCH], dt.float32, tag="dtb")
                    nc.sync.dma_start(dtb[:, :], ins[f"{d}_dtb"][:, :])
                    del_big = ppre.tile([128, NCH * T], dt.float16, tag="delbig" + d)
                    for ch in range(NCH):
                        dp = psB.tile([128, 512], dt.float32, tag="dpps")
                        nc.tensor.matmul(dp[:, :],
                                         dtwT[:, ch * 128:(ch + 1) * 128],
                                         dt_sb[:, :], start=True, stop=True)
                        et = ppw.tile([128, 512], dt.float32, tag="et")
                        nc.scalar.activation(et[:, :], dp[:, :], AF.Exp,
                                             bias=dtb[:, ch:ch + 1])
                        nc.scalar.activation(del_big[:, ch * T:(ch + 1) * T],
                                             et[:, :], AF.Ln, bias=1.0)
                    # u = delta * silu(xconv)   (bf16)
                    u_big = ppre.tile([128, NCH * T], dt.bfloat16, tag="ubig" + d)
                    for ch in range(NCH):
                        cs = slice(ch * T, (ch + 1) * T)
                        nc.vector.tensor_tensor(u_big[:, cs], del_big[:, cs],
                                                sx_big[:, cs], OP.mult)
                    # sxd = sx * D * 0.25  (bf16)
                    Dt = ppw.tile([128, NCH], dt.float32, tag="Dt")
                    nc.sync.dma_start(Dt[:, :], ins[f"{d}_D"][:, :])
                    sxd_big = ppre.tile([128, NCH * T], dt.bfloat16, tag="sxdbig" + d)
                    for ch in range(NCH):
                        nc.vector.tensor_scalar(
                            sxd_big[:, ch * T:(ch + 1) * T],
                            sx_big[:, ch * T:(ch + 1) * T],
                            Dt[:, ch:ch + 1], 0.25, OP.mult, op1=OP.mult)

                    # --- stage AG payloads ---
                    for ch in range(NCH):
                        cs = slice(ch * T, (ch + 1) * T)
                        nc.sync.dma_start(agt[f"d_in_{d}"][:, cs],
                                          del_big[:, cs])
                        nc.scalar.dma_start(agt[f"u_in_{d}"][:, cs],
                                            u_big[:, cs])
                    nc.sync.dma_start(agt[f"g_in_{d}"][0:128, :], sz_big[:, :])
                    nc.sync.dma_start(agt[f"g_in_{d}"][128:256, :], sxd_big[:, :])

            # ================= collectives: reshard ====================
            OPb = mybir.AluOpType.bypass
            for d in ("f", "b"):
                for nm in ("bc", "d", "u", "g"):
                    nc.gpsimd.collective_compute(
                        "AllGather", OPb, replica_groups=groups,
                        ins=[agt[f"{nm}_in_{d}"][:, :]],
                        outs=[agt[f"{nm}_out_{d}"][:, :]])

            # ================= scan phase (my 4 n's, full T) ===========
            for d in ("f", "b"):
                roff = 0 if d == "f" else 128
                boff = 0 if d == "f" else 32
                with tc.tile_pool(name=f"scan{d}", bufs=1) as psc, \
                     tc.tile_pool(name=f"scw{d}", bufs=2) as pscw:
                    # full-T B^T / C^T ([16, L]) assembled from quarters
                    BT = psc.tile([16, L], dt.bfloat16, tag="BT")
                    CT = psc.tile([16, L], dt.bfloat16, tag="CT")
                    for qq in range(4):
                        nc.sync.dma_start(
                            BT[:, qq * T:(qq + 1) * T],
                            ag_bc_out[qq * 64 + boff:qq * 64 + boff + 16, :])
                        nc.sync.dma_start(
                            CT[:, qq * T:(qq + 1) * T],
                            ag_bc_out[qq * 64 + boff + 16:qq * 64 + boff + 32, :])
                    # replicate my 4 n-rows across partitions (PE)
                    Brep, Crep = [], []
                    with tc.tile_pool(name=f"rp{d}", bufs=2,
                                      space="PSUM") as prep:
                        for ni in range(4):
                            for (srct, dst) in ((BT, Brep), (CT, Crep)):
                                rp = psc.tile(
                                    [128, L], dt.bfloat16,
                                    tag=f"rep{'B' if dst is Brep else 'C'}{ni}",
                                    name=f"rep{d}{ni}{len(dst)}")
                                for fc in range(4):
                                    pr = prep.tile([128, 512], dt.float32,
                                                   tag="repps")
                                    nc.tensor.matmul(
                                        pr[:, :],
                                        repsel[:, ni * 128:(ni + 1) * 128],
                                        srct[:, fc * 512:(fc + 1) * 512],
                                        start=True, stop=True)
                                    nc.vector.tensor_copy(
                                        rp[:, fc * 512:(fc + 1) * 512], pr[:, :])
                                dst.append(rp)
                    ascale = psc.tile([128, NCH * 4], dt.float32, tag="ascale")
                    nc.sync.dma_start(ascale[:, :], ins[f"{d}_ascale"][:, :])
                    pyps_cm = tc.tile_pool(name=f"yps{d}", bufs=2,
                                           space="PSUM")
                    pyps = pyps_cm.__enter__()
                    for ch in range(NCH):
                        # stream full-T delta / u / sz / sxd for this chunk
                        dl = pscw.tile([128, L], dt.float16, tag="dl")
                        uu = pscw.tile([128, L], dt.bfloat16, tag="uu")
                        sz = pscw.tile([128, L], dt.bfloat16, tag="sz")
                        sxd = pscw.tile([128, L], dt.bfloat16, tag="sxd")
                        for qq in range(4):
                            r0 = qq * 256 + roff
                            g0 = qq * 512 + (0 if d == "f" else 256)
                            cs = slice(ch * T, (ch + 1) * T)
                            ts_ = slice(qq * T, (qq + 1) * T)
                            nc.sync.dma_start(dl[:, ts_], ag_d_out[r0:r0 + 128, cs])
                            nc.sync.dma_start(uu[:, ts_], ag_u_out[r0:r0 + 128, cs])
                            nc.sync.dma_start(sz[:, ts_],
                                              ag_g_out[g0:g0 + 128, cs])
                            nc.sync.dma_start(sxd[:, ts_],
                                              ag_g_out[g0 + 128:g0 + 256, cs])
                        y_ps = pyps.tile([128, L], dt.float32, tag="yps")
                        for ni in range(4):
                            dA = pscw.tile([128, L], dt.float32, tag="dA")
                            nc.scalar.activation(dA[:, :], dl[:, :], AF.Exp,
                                                 scale=ascale[:, ch * 4 + ni:
                                                              ch * 4 + ni + 1])
                            dBx = pscw.tile([128, L], dt.bfloat16, tag="dBx")
                            nc.vector.tensor_tensor(dBx[:, :], uu[:, :],
                                                    Brep[ni][:, :], OP.mult)
                            hh = pscw.tile([128, L], dt.bfloat16, tag="hh")
                            nc.vector.tensor_tensor_scan(
                                hh[:, :], dA[:, :], dBx[:, :], 0.0,
                                OP.mult, OP.add)
                            hC = pscw.tile([128, L], dt.bfloat16, tag="hC")
                            nc.vector.tensor_tensor(hC[:, :], hh[:, :],
                                                    Crep[ni][:, :], OP.mult)
                            for fc in range(4):
                                nc.tensor.matmul(
                                    y_ps[:, fc * 512:(fc + 1) * 512],
                                    ident[:, :], hC[:, fc * 512:(fc + 1) * 512],
                                    start=(ni == 0), stop=False)
                        # + sxd/4 via identity matmul, then gate with silu(z)
                        for fc in range(4):
                            nc.tensor.matmul(
                                y_ps[:, fc * 512:(fc + 1) * 512], ident[:, :],
                                sxd[:, fc * 512:(fc + 1) * 512],
                                start=False, stop=True)
                        dst = yg[(d, ch)][:, :]
                        if d == "b":
                            dst = dst[:, ::-1]
                        nc.vector.tensor_tensor(dst, y_ps[:, :], sz[:, :], OP.mult)
                    pyps_cm.__exit__(None, None, None)

            # ================= output matmul + ReduceScatter ===========
            with tc.tile_pool(name="outp", bufs=2) as pout, \
                 tc.tile_pool(name="outps", bufs=2, space="PSUM") as pops:
                W2 = {}
                for d in ("f", "b"):
                    W2[d] = pout.tile([128, NCH * DIM], dt.bfloat16, tag=f"W2{d}", name=f"W2{d}")
                    for ch in range(NCH):
                        nc.sync.dma_start(
                            W2[d][:, ch * DIM:(ch + 1) * DIM],
                            ins[f"{d}_W2T"][ch * 128:(ch + 1) * 128, :])
                for m in range(16):
                    ps = pops.tile([128, DIM], dt.float32, tag="ops")
                    first = True
                    for d in ("f", "b"):
                        for ch in range(NCH):
                            nc.tensor.matmul(
                                ps[:, :],
                                yg[(d, ch)][:, m * 128:(m + 1) * 128],
                                W2[d][:, ch * DIM:(ch + 1) * DIM],
                                start=first, stop=(d == "b" and ch == NCH - 1))
                            first = False
                    ob = pout.tile([128, DIM], dt.float32, tag="ob")
                    nc.vector.tensor_copy(ob[:, :], ps[:, :])
                    nc.sync.dma_start(rs_in[m * 128:(m + 1) * 128, :], ob[:, :])
                nc.gpsimd.collective_compute(
                    "ReduceScatter", mybir.AluOpType.add, replica_groups=groups,
                    ins=[rs_in[:, :]], outs=[rs_out[:, :]])
                for i in range(4):
                    rt = pout.tile([128, DIM], dt.float32, tag="rt")
                    nc.sync.dma_start(rt[:, :], rs_out[i * 128:(i + 1) * 128, :])
                    xr = pout.tile([128, DIM], dt.float32, tag="xr")
                    nc.sync.dma_start(xr[:, :], xres_in[i * 128:(i + 1) * 128, :])
                    oo = pout.tile([128, DIM], dt.float32, tag="oo")
                    nc.vector.tensor_tensor(oo[:, :], rt[:, :], xr[:, :], OP.add)
                    nc.sync.dma_start(out_t[i * 128:(i + 1) * 128, :], oo[:, :])

    nc.compile()
    return nc


def _prep_host(inputs):
    """Fold weights, build the 8 per-core input maps."""
    f32 = np.float32
    x = np.asarray(inputs["x"], f32)
    ln_g = np.asarray(inputs["ln_g"], f32)
    ln_b = np.asarray(inputs["ln_b"], f32)
    fuse_w = np.asarray(inputs["fuse_w"], f32)
    fuse_b = np.asarray(inputs["fuse_b"], f32)

    shared = {}
    shared["ident"] = _bf16(np.eye(128))
    for d in ("f", "b"):
        P = {k[2:]: np.asarray(v, f32) for k, v in inputs.items()
             if k.startswith(d + "_")}
        inW = P["inproj"] * ln_g[None, :]
        shared[f"{d}_inT"] = _bf16(inW.T)                       # [512, 2048]
        shared[f"{d}_inb"] = (P["inproj"] @ ln_b).reshape(16, 128).T.copy()
        shared[f"{d}_convw"] = P["convw"].reshape(NCH, 128, 4).transpose(
            1, 0, 2).reshape(128, NCH * 4).copy()
        shared[f"{d}_convb"] = P["convb"].reshape(NCH, 128).T.copy()
        shared[f"{d}_xprojT"] = _bf16(P["xproj"].T)             # [1024, 64]
        shared[f"{d}_dtwT"] = _bf16(P["dtw"].T)                 # [32, 1024]
        shared[f"{d}_dtb"] = P["dtb"].reshape(NCH, 128).T.copy()
        shared[f"{d}_D"] = P["D"].reshape(NCH, 128).T.copy()
        shared[f"{d}_W2T"] = _bf16((fuse_w @ P["outw"]).T)      # [1024, 512]
        shared[f"{d}_A"] = -np.exp(P["Alog"])                   # [1024, 16]

    xr_full = x[:, ::-1, :]
    maps = []
    for c in range(NCORES):
        b, g = c // 4, c % 4
        m = {k: v for k, v in shared.items() if not k.endswith("_A")}

        def pad_slice(src):
            lo = g * T - 3
            sl = np.zeros((640, DIM), f32)
            a = max(lo, 0)
            sl[a - lo:(g + 1) * T - lo, :] = src[a:(g + 1) * T, :]
            return sl

        m["xf"] = pad_slice(x[b])
        m["xb"] = pad_slice(xr_full[b])
        m["xres"] = (x[b, g * T:(g + 1) * T, :] + fuse_b[None, :]).astype(f32)
        rs = np.zeros((16, 4 * 128), f32)
        for ni in range(4):
            rs[4 * g + ni, ni * 128:(ni + 1) * 128] = 1.0
        m["repsel"] = _bf16(rs)
        for d in ("f", "b"):
            A = shared[f"{d}_A"]  # [1024, 16]
            asc = np.zeros((128, NCH * 4), f32)
            for ch in range(NCH):
                for ni in range(4):
                    asc[:, ch * 4 + ni] = A[ch * 128:(ch + 1) * 128, 4 * g + ni]
            m[f"{d}_ascale"] = asc
        m["xf"] = m["xf"].astype(f32)
        maps.append(m)
    return maps


def kernel(**inputs):
    from concourse.bass_utils import run_bass_kernel_spmd

    if "nc" not in _CACHE:
        _CACHE["nc"] = _build_program()
    nc = _CACHE["nc"]

    maps = _prep_host(inputs)
    res = run_bass_kernel_spmd(nc, maps, core_ids=list(range(NCORES)))
    out = np.zeros((BSZ, L, DIM), np.float32)
    for c in range(NCORES):
        b, g = c // 4, c % 4
        out[b, g * T:(g + 1) * T, :] = res.results[c]["out"]
    return out
